# revision 48
# baseline (speedup 1.0000x reference)
"""Trainium2 Bass kernel for BaseMessageModule (GNN message passing).

Strategy (host-gathered embedding stream + device one-hots at DVE 2x,
3-stage software pipeline across super-pairs):
- Shard ATOMS across the 8 cores (3750 each). Host routes each pair to the
  core owning its receiving atom idx_i; pairs are sorted by TILE key.
- Atoms are BIN-PACKED (first-fit-decreasing, non-contiguous) into tiles
  of <= 16 atoms and exactly CPT*128 pair slots (~97% slot fill). Tile
  t's pairs accumulate into a PSUM slice addressed by t (static
  schedule, SPMD-safe).
- The neighbor-embedding gather E[idx_j] is done on the HOST (pure integer
  indexing, like the routing/sorting): rows are shipped pre-ordered per
  pair slot as one contiguous bf16 stream, consumed with one plain 1MB
  DMA per super-pair at the measured HBM ceiling (v1's per-row SWDGE
  descriptor generation was the 219us critical path).
- Key linearity: segment-sum first, then apply W once per atom (20x less
  matmul), bias as count[n] * b.
- One-hot coefficient planes built ON DEVICE from per-chunk slot indices:
  cmp2 = is_equal(iota, ii) and O~ = cmp2 * Cu4 (Cu4 = [f, f*u0, f*u1,
  f*u2] from f/r). Layout (chunk, atom, k) with k innermost and x2-
  duplicated indices keeps every DVE operand innermost-stride-1 (2x mode)
  AND keeps each chunk's 64 rhs columns contiguous (65ns/MM vs 232ns
  strided). Builds run 2 pairs ahead of the matmuls.
- Per 128-pair chunk: PSUM[f, (a,k)] += E_chunk.T @ O~_chunk.
- Ramp: a tiny duplicate head tensor (ghd) lands early and gates only a
  half-width coefficient chain, so pair 0's matmuls start ~17us in.
  INVARIANT: every Cu4i column must be WRITTEN (program order) before
  the build that reads it is emitted -- head-A covers builds 0/1, head-B
  (emitted before the loop) covers builds 2/3, the tail spans (emitted
  at loop iter 0, before build 4) cover the rest. Violating this reads
  uninitialized SBUF (NaNs).
- Tail, 3-stage pipelined: [iter b] chunk matmuls; U + radial drained on
  scalar to free PSUM; [iter b+1] W-transform + count*b bias on PE,
  squares on scalar from PSUM, norm-adds on GPSIMD (vector for the last
  pairs: they sit on the serial drain); [iter b+2] sqrt + store on the
  scalar HWDGE ring. Output stays [f, slot]-major bf16; the HOST does
  the final transpose (pure layout).

All floating-point arithmetic runs on device. Host work is integer index
manipulation (routing/sorting/padding/gather = sharding) and array layout.
"""

from contextlib import ExitStack

import ml_dtypes
import numpy as np

import concourse.bass as bass
import concourse.bacc as bacc
import concourse.tile as tile
from concourse import mybir
from concourse.bass_utils import run_bass_kernel_spmd

F = 128
ATILE = 16  # atom window per tile
KBLK = 4  # coefficient planes: radial, u0, u1, u2
CHUNK = 128  # pairs per matmul chunk
CPT = 2  # chunks (of 128 pair slots) per tile
TPS = 8  # tiles per super
SUP_C = TPS * CPT  # chunks per super (16)
EW = SUP_C * F  # E cols per super (2048)
PF = 6  # E-stream prefetch depth (supers in flight)

TW = ATILE * KBLK  # one-hot cols per chunk / psum cols per tile (64)


def _ap(t_ap, free_dims, off=0):
    """Custom AP view over the same partitions as t_ap with given free dims."""
    return bass.AP(t_ap.tensor, t_ap.offset + off, [t_ap.ap[0]] + list(free_dims))


def build_nc(N, T, n_cores):
    """Build the SPMD program for one core with T pair tiles."""
    CH = T * CPT  # chunks per core
    n_super = T // TPS
    n_sp = T // (2 * TPS)  # super-pairs (tail granularity)
    BW2 = 2 * TPS * 3 * ATILE  # u-plane cols per super-pair (768)
    C2 = 2 * SUP_C  # chunks per super-pair (32)

    fp = mybir.dt.float32
    bf = mybir.dt.bfloat16

    nc = bacc.Bacc("TRN2", target_bir_lowering=False, debug=False,
                   num_devices=n_cores)

    egd = nc.dram_tensor("egd", [128, n_super * EW], bf, kind="ExternalInput")
    gind = nc.dram_tensor("gind", [128, 6 * CH], bf, kind="ExternalInput")
    # duplicate head slice of gind (first HEAD chunks): tiny, lands ~3.5us
    # earlier than the full gind, ungating the phase-1 DVE chain
    HEAD = min(4 * C2, CH)
    ghd = nc.dram_tensor("ghd", [128, 6 * HEAD], bf, kind="ExternalInput")
    c3d = nc.dram_tensor("c3d", [1, n_sp * BW2], bf, kind="ExternalInput")
    wTd = nc.dram_tensor("wTd", [F, F], fp, kind="ExternalInput")
    browd = nc.dram_tensor("browd", [1, F], fp, kind="ExternalInput")
    outd = nc.dram_tensor("outd", [128, n_sp * 512], bf,
                          kind="ExternalOutput")

    mult, add = mybir.AluOpType.mult, mybir.AluOpType.add
    iseq = mybir.AluOpType.is_equal

    with tile.TileContext(nc) as tc, ExitStack() as ctx:
        cpool = ctx.enter_context(tc.tile_pool(name="const", bufs=1))
        mpool = ctx.enter_context(tc.tile_pool(name="main", bufs=1))

        with tc.tile_pool(name="esup", bufs=PF // 2 + 1) as epool, \
             tc.tile_pool(name="cmp", bufs=3) as cmpool, \
             tc.tile_pool(name="ohp", bufs=3) as opool, \
             tc.tile_pool(name="pacc", bufs=3, space="PSUM") as ppool, \
             tc.tile_pool(name="pw", bufs=1, space="PSUM") as wpool, \
             tc.tile_pool(name="u", bufs=3) as upool, \
             tc.tile_pool(name="sq", bufs=3) as sqpool, \
             tc.tile_pool(name="s0", bufs=3) as s0pool, \
             tc.tile_pool(name="ob", bufs=3) as obpool:

            # --- constants + phase-1 inputs first: these small DMAs gate
            # the one-hot pipeline, so they go ahead of the big E stream ---
            # iota over (a, k2): value = a  (exact in bf16 for 0..15)
            iotad = cpool.tile([128, 2 * ATILE], bf)
            nc.gpsimd.iota(_ap(iotad[:], [[2, ATILE], [1, 2]]),
                           [[1, ATILE], [0, 2]],
                           channel_multiplier=0,
                           allow_small_or_imprecise_dtypes=True)
            # head inputs first (164KB, lands ~7.5us), then the rest in
            # ONE ~1MB DMA (five separate small DMAs cost ~3us of
            # serialized issue + staggered arrival)
            gh = cpool.tile([128, 6 * HEAD], bf)
            nc.sync.dma_start(out=gh[:], in_=ghd[:])
            gin = cpool.tile([128, 6 * CH], bf)
            nc.sync.dma_start(out=gin[:], in_=gind[:])
            ii_sb = gin     # cols [0, 2CH)
            FOFF, R0, R1, R2 = 2 * CH, 3 * CH, 4 * CH, 5 * CH
            HF, H0, H1, H2 = 2 * HEAD, 3 * HEAD, 4 * HEAD, 5 * HEAD
            e_tiles = {}

            def prefetch(bp):
                # one 1MB DMA per super-pair: 512KB transfers run at ~75%
                # of the 1MB-class DMA rate, so batching both supers buys
                # ~15% effective input bandwidth
                e = epool.tile([128, 2 * EW], bf, tag="esup")
                nc.sync.dma_start(out=e[:],
                                  in_=egd[:, bp * 2 * EW:(bp + 1) * 2 * EW])
                e_tiles[bp] = e

            prefetch(0)  # pair 0's E rides right behind the gating inputs

            wT_sb = cpool.tile([F, F], fp)
            nc.sync.dma_start(out=wT_sb[:], in_=wTd[:])
            wT_bf = cpool.tile([F, F], bf)
            nc.scalar.copy(wT_bf[:], wT_sb[:])
            brow_sb = cpool.tile([1, F], fp)
            nc.sync.dma_start(out=brow_sb[:], in_=browd[:])
            brow_bf = cpool.tile([1, F], bf)
            nc.scalar.copy(brow_bf[:], brow_sb[:])
            c3sb = cpool.tile([1, n_sp * BW2], bf)
            nc.sync.dma_start(out=c3sb[:], in_=c3d[:])

            # rest of the E prefetch window
            for bp in range(1, min(PF // 2, n_sp)):
                prefetch(bp)

            # --- Phase 1: f (bf16), |r|, 1/|r|, f/|r|, Cu4i planes ---
            # produced in a small head span (first 4 pairs) + the
            # remainder, so pair 0's build is gated by ~3us of DVE work
            # instead of the full-width chain (incl the 4.4us reciprocal)
            Cu4i = mpool.tile([128, KBLK * CH], bf)
            fb = mpool.tile([128, CH], bf)
            tA = mpool.tile([128, CH], bf)
            tB = mpool.tile([128, CH], bf)

            def coeff_span(ch0, ch1, src=None, fo=None, o0=None, o1=None,
                           o2=None):
                if src is None:
                    src, fo, o0, o1, o2 = gin, FOFF, R0, R1, R2
                s_ = slice(ch0, ch1)
                nc.vector.tensor_copy(fb[:, s_],
                                      src[:, fo + ch0:fo + ch1])
                nc.vector.tensor_tensor(out=tA[:, s_],
                                        in0=src[:, o0 + ch0:o0 + ch1],
                                        in1=src[:, o0 + ch0:o0 + ch1],
                                        op=mult)
                nc.vector.tensor_tensor(out=tB[:, s_],
                                        in0=src[:, o1 + ch0:o1 + ch1],
                                        in1=src[:, o1 + ch0:o1 + ch1],
                                        op=mult)
                nc.vector.tensor_tensor(out=tA[:, s_], in0=tA[:, s_],
                                        in1=tB[:, s_], op=add)
                nc.vector.tensor_tensor(out=tB[:, s_],
                                        in0=src[:, o2 + ch0:o2 + ch1],
                                        in1=src[:, o2 + ch0:o2 + ch1],
                                        op=mult)
                nc.vector.tensor_tensor(out=tA[:, s_], in0=tA[:, s_],
                                        in1=tB[:, s_], op=add)
                nc.scalar.sqrt(tA[:, s_], tA[:, s_])  # |r|
                with nc.allow_low_precision(reason="1/|r| feeds bf16 one-hots"):
                    nc.vector.reciprocal(tB[:, s_], tA[:, s_])
                nc.vector.tensor_tensor(out=tB[:, s_], in0=fb[:, s_],
                                        in1=tB[:, s_], op=mult)

            def cu_span(ch0, ch1):
                w = ch1 - ch0
                ksl = [[KBLK, w]]
                o4 = ch0 * KBLK
                nc.vector.tensor_copy(_ap(Cu4i[:], ksl, off=o4),
                                      fb[:, ch0:ch1])
                src, o0, o1, o2 = ((gh, H0, H1, H2) if ch1 <= HEAD
                                   else (gin, R0, R1, R2))
                nc.vector.tensor_tensor(
                    out=_ap(Cu4i[:], ksl, off=o4 + 1),
                    in0=tB[:, ch0:ch1],
                    in1=src[:, o0 + ch0:o0 + ch1], op=mult)
                nc.vector.tensor_tensor(
                    out=_ap(Cu4i[:], ksl, off=o4 + 2),
                    in0=tB[:, ch0:ch1],
                    in1=src[:, o1 + ch0:o1 + ch1], op=mult)
                nc.vector.tensor_tensor(
                    out=_ap(Cu4i[:], ksl, off=o4 + 3),
                    in0=tB[:, ch0:ch1],
                    in1=src[:, o2 + ch0:o2 + ch1], op=mult)

            # head chain in two halves: builds 0/1 only need the first
            # 2*C2 chunks, so don't gate them on the full-head reciprocal;
            # head-B still lands before build(2) is emitted (loop iter 0),
            # preserving the Cu4i coverage invariant
            H2W = min(2 * C2, HEAD)
            coeff_span(0, H2W, gh, HF, H0, H1, H2)
            cu_span(0, H2W)

            def build(b):
                """One-hot (cmp + 2 mults) for super-pair b."""
                ch0 = b * C2
                o4 = ch0 * KBLK
                # cmp2[p, (ch, a, k2)] = (iota[a] == ii[ch]), x2-duplicated
                cmp = cmpool.tile([128, C2 * ATILE * 2], bf, tag="cmp")
                nc.vector.tensor_tensor(
                    out=_ap(cmp[:], [[2 * ATILE, C2], [2, ATILE], [1, 2]]),
                    in0=_ap(iotad[:], [[0, C2], [2, ATILE], [1, 2]]),
                    in1=_ap((gh if (ch0 + C2) * 2 <= 2 * HEAD
                             else ii_sb)[:],
                            [[2, C2], [0, ATILE], [1, 2]],
                            off=ch0 * 2),
                    op=iseq,
                )
                # O~[p, (ch, a, k)] = cmp2 * Cu4i[ch, k]  (two k-pair passes)
                oh = opool.tile([128, C2 * TW], bf, tag="oh")
                for h in range(2):
                    nc.vector.tensor_tensor(
                        out=_ap(oh[:], [[TW, C2], [KBLK, ATILE], [1, 2]],
                                off=2 * h),
                        in0=_ap(cmp[:], [[2 * ATILE, C2], [2, ATILE], [1, 2]]),
                        in1=_ap(Cu4i[:], [[KBLK, C2], [0, ATILE], [1, 2]],
                                off=o4 + 2 * h),
                        op=mult,
                    )
                return oh

            oh_tiles = {0: build(0)}
            if n_sp > 1:
                oh_tiles[1] = build(1)
            if H2W < HEAD:
                coeff_span(H2W, HEAD, gh, HF, H0, H1, H2)
                cu_span(H2W, HEAD)

            pend = {}   # b -> (U, ob) awaiting stage-1 finish
            pend2 = {}  # b -> (ob, s0t) awaiting stage-2 finish

            def finish1(b):
                """W transform + squares + norm-adds for pair b (one pair
                late: its PE work rides right behind the chunk-MM stream,
                its scalar work fills the W-matmul window)."""
                U, ob = pend.pop(b)
                pw = wpool.tile([128, 1024], fp, tag="pw")
                for q in range(4):
                    p0, u0 = q * 256, q * 192
                    nc.tensor.matmul(out=pw[:, p0:p0 + 192], lhsT=wT_bf[:],
                                     rhs=U[:, u0:u0 + 192],
                                     start=True, stop=False)
                    nc.tensor.matmul(out=pw[:, p0:p0 + 192],
                                     lhsT=brow_bf[:1, :],
                                     rhs=c3sb[:1, b * BW2 + u0:
                                              b * BW2 + u0 + 192],
                                     start=False, stop=True)
                sq = sqpool.tile([128, BW2], bf, tag="sq")
                nc.scalar.activation(
                    _ap(sq[:], [[192, 4], [1, 192]]),
                    _ap(pw[:], [[256, 4], [1, 192]]),
                    mybir.ActivationFunctionType.Square)
                # sq cols are (t, k1..3, a): sum the 3 planes per (t, a);
                # vector engine for the last pairs (gpsimd adds are 3x
                # slower and sit on the serial end-of-kernel drain chain)
                eng = nc.vector if b >= n_sp - 2 else nc.gpsimd
                s0t = s0pool.tile([128, 256], bf, tag="s0")
                eng.tensor_tensor(
                    out=_ap(s0t[:], [[ATILE, 16], [1, ATILE]]),
                    in0=_ap(sq[:], [[3 * ATILE, 16], [1, ATILE]]),
                    in1=_ap(sq[:], [[3 * ATILE, 16], [1, ATILE]], off=ATILE),
                    op=add,
                )
                eng.tensor_tensor(
                    out=_ap(s0t[:], [[ATILE, 16], [1, ATILE]]),
                    in0=_ap(s0t[:], [[ATILE, 16], [1, ATILE]]),
                    in1=_ap(sq[:], [[3 * ATILE, 16], [1, ATILE]],
                            off=2 * ATILE),
                    op=add,
                )
                pend2[b] = (ob, s0t)

            def finish2(b):
                """sqrt + store for pair b (two pairs late so the sqrt's
                wait on the gpsimd adds never blocks the scalar FIFO)."""
                ob, s0t = pend2.pop(b)
                nc.scalar.sqrt(ob[:, 0:256], s0t[:])
                # store on the scalar HWDGE ring so its wait never
                # head-of-line-blocks the sync ring's E stream
                nc.scalar.dma_start(out=outd[:, b * 512:(b + 1) * 512],
                                    in_=ob[:])

            for b in range(n_sp):
                if b + PF // 2 < n_sp:
                    prefetch(b + PF // 2)
                oh = oh_tiles.pop(b)
                e = e_tiles.pop(b)

                # segment-sum matmuls: 16 tiles into one 2-bank PSUM tile
                acc2 = ppool.tile([128, 2 * TPS * TW], fp, tag="acc")
                for s2 in range(2):
                    for ti in range(TPS):
                        for ci in range(CPT):
                            cp = s2 * SUP_C + ti * CPT + ci
                            nc.tensor.matmul(
                                out=acc2[:, s2 * 512 + ti * TW:
                                         s2 * 512 + (ti + 1) * TW],
                                lhsT=_ap(e[:], [[1, F]], off=cp * F),
                                rhs=_ap(oh[:], [[1, TW]], off=cp * TW),
                                start=(ci == 0),
                                stop=(ci == CPT - 1),
                            )

                # build pair b+2 before pair b's tail so the DVE FIFO never
                # gates pair b+1/b+2's matmuls behind tail dependencies
                if b + 2 < n_sp:
                    oh_tiles[b + 2] = build(b + 2)
                if b == 0 and HEAD < CH:
                    coeff_span(HEAD, CH)
                    cu_span(HEAD, CH)

                # finish1 BEFORE the U/R drain: the W-matmuls' PSUM-write
                # guard keys on scalar PSUM-reads emitted before them, so
                # emitting U_b/R_b first would chain pair b-1's W-MMs to
                # pair b's U copy (measured 1.9us/pair stall)
                if b > 0:
                    finish1(b - 1)

                # drain acc2 (U on scalar into (t,k,a) order, radial on
                # scalar) to free the PSUM buffer
                U = upool.tile([128, BW2], bf, tag="u")
                for q in range(4):
                    nc.scalar.copy(
                        _ap(U[:], [[3 * ATILE, 4], [ATILE, 3], [1, ATILE]],
                            off=q * 4 * 3 * ATILE),
                        _ap(acc2[:], [[TW, 4], [1, 3], [KBLK, ATILE]],
                            off=q * 4 * TW + 1))
                ob = obpool.tile([128, 512], bf, tag="ob")
                nc.scalar.copy(
                    _ap(ob[:], [[ATILE, 16], [1, ATILE]], off=256),
                    _ap(acc2[:], [[TW, 16], [KBLK, ATILE]]))
                pend[b] = (U, ob)

                if b > 1:
                    finish2(b - 2)
            finish1(n_sp - 1)
            finish2(n_sp - 2)
            finish2(n_sp - 1)

    nc.compile()
    return nc


def host_prep(inputs, n_cores=8):
    """Route pairs to atom-owning cores; variable-base 16-atom pair tiles."""
    emb = np.ascontiguousarray(np.asarray(inputs["atomic_embedding"],
                                          dtype=np.float32))
    # ship the high 16 bits of each fp32 (== the bf16 bit pattern, truncated):
    # a pure byte-slice of the input, no host arithmetic
    embh = np.ascontiguousarray(
        emb.view(np.uint16).reshape(emb.shape[0], -1)[:, 1::2]
    ).view(ml_dtypes.bfloat16)
    f = np.asarray(inputs["f_ij_cutoff"], dtype=np.float32).ravel()
    r = np.asarray(inputs["r_ij"], dtype=np.float32)
    W = np.asarray(inputs["W"], dtype=np.float32)
    b = np.asarray(inputs["b"], dtype=np.float32)
    pl = np.asarray(inputs["pairlist"]).astype(np.int64)
    idx_i, idx_j = pl[0], pl[1]

    N = emb.shape[0]
    P = idx_i.shape[0]
    APC = N // n_cores
    SLOTS = CPT * CHUNK  # pair slots per tile (256)

    cnt_atom = np.bincount(idx_i, minlength=N).astype(np.int64)

    # bin-pack atoms into tiles (<= ATILE atoms, <= SLOTS pairs each);
    # atoms need not be contiguous: take the heaviest remaining atom,
    # then fill from the light end (two-pointer) for ~98% slot fill
    tiles_atoms = []  # per core: list of lists of atom ids (global)
    tile_of_atom = np.zeros(N, dtype=np.int64)
    slot_of_atom = np.zeros(N, dtype=np.int64)
    for c in range(n_cores):
        ca = cnt_atom[c * APC:(c + 1) * APC]
        order_d = np.argsort(-ca, kind="stable")
        bins = []        # atom lists
        bin_p = []       # slots used
        for a in order_d:
            d = int(ca[a])
            for t in range(len(bins)):
                if bin_p[t] + d <= SLOTS and len(bins[t]) < ATILE:
                    bins[t].append(a)
                    bin_p[t] += d
                    break
            else:
                bins.append([a])
                bin_p.append(d)
        for t, cur in enumerate(bins):
            for s_, a in enumerate(cur):
                tile_of_atom[c * APC + a] = t
                slot_of_atom[c * APC + a] = s_
        tiles_atoms.append([[c * APC + a for a in cur] for cur in bins])
    T = max(len(tl) for tl in tiles_atoms)
    T = ((T + 15) // 16) * 16  # multiple of 16 for super-pair tails

    # sort pairs by TILE KEY (not by atom): the starts/pos slot arithmetic
    # below needs same-key pairs contiguous, and with bin-packed tiles the
    # atom -> tile map is non-monotone
    pkey = (idx_i // APC) * T + tile_of_atom[idx_i]
    order = np.argsort(pkey, kind="stable")
    so_i = idx_i[order]
    core_of = so_i // APC
    key = pkey[order]
    cnt = np.bincount(key, minlength=n_cores * T)
    assert cnt.max() <= SLOTS, cnt.max()
    starts = np.zeros(n_cores * T + 1, dtype=np.int64)
    np.cumsum(cnt, out=starts[1:])
    pos = np.arange(P, dtype=np.int64) - starts[key]
    slot = key * SLOTS + pos
    TOT = n_cores * T * SLOTS

    jj = np.zeros(TOT, dtype=np.int32)  # pad slots: row 0 (one-hot kills it)
    ff = np.zeros(TOT, dtype=np.float32)
    rr = np.zeros((TOT, 3), dtype=np.float32)
    rr[:, 0] = 1.0
    ii = np.full(TOT, 255, dtype=np.int32)  # pad slots: no atom slot
    jj[slot] = idx_j[order]
    ff[slot] = f[order]
    rr[slot] = r[order]
    ii[slot] = slot_of_atom[so_i]

    CH = T * CPT
    n_sp = T // (2 * TPS)
    BW2 = 2 * TPS * 3 * ATILE
    TOTC = T * SLOTS
    in_maps = []
    out_sel = []  # per core: (valid slot rows, global atom rows)
    for c in range(n_cores):
        sl = slice(c * TOTC, (c + 1) * TOTC)
        tr = lambda x: np.ascontiguousarray(x.reshape(CH, CHUNK).T)
        # host-side gather of neighbor embedding rows, pair-slot order,
        # laid out [pair-in-chunk, (chunk, f)]
        jj_c = jj[sl].reshape(CH, CHUNK)
        eg = np.ascontiguousarray(
            embh[jj_c].transpose(1, 0, 2).reshape(CHUNK, CH * F))
        # slot indices, x2-duplicated: [p, (ch, j2)]
        ii_c = tr(ii[sl].astype(np.float32))  # [128, CH]
        ii2 = np.ascontiguousarray(
            np.repeat(ii_c, 2, axis=1)).astype(ml_dtypes.bfloat16)
        # counts per (super-pair, tile, k-plane, atom)
        cnt3 = np.zeros((T, 3, ATILE), dtype=np.float32)
        rows_slot = []
        rows_atom = []
        for t, atoms in enumerate(tiles_atoms[c]):
            aa = np.asarray(atoms, dtype=np.int64)
            cnt3[t, :, :len(aa)] = cnt_atom[aa][None, :]
            rows_slot.append(t * ATILE + np.arange(len(aa)))
            rows_atom.append(aa)
        out_sel.append((np.concatenate(rows_slot), np.concatenate(rows_atom)))
        gind = np.concatenate(
            [ii2,
             tr(ff[sl]).astype(ml_dtypes.bfloat16),
             tr(rr[sl][:, 0]).astype(ml_dtypes.bfloat16),
             tr(rr[sl][:, 1]).astype(ml_dtypes.bfloat16),
             tr(rr[sl][:, 2]).astype(ml_dtypes.bfloat16)], axis=1)
        HEAD = min(4 * 2 * SUP_C, CH)
        ghd = np.concatenate(
            [gind[:, 0:2 * HEAD],
             gind[:, 2 * CH:2 * CH + HEAD],
             gind[:, 3 * CH:3 * CH + HEAD],
             gind[:, 4 * CH:4 * CH + HEAD],
             gind[:, 5 * CH:5 * CH + HEAD]], axis=1)
        in_maps.append({
            "egd": eg,
            "gind": np.ascontiguousarray(gind),
            "ghd": np.ascontiguousarray(ghd),
            "c3d": np.ascontiguousarray(
                cnt3.reshape(n_sp, BW2).reshape(1, -1)).astype(
                    ml_dtypes.bfloat16),
            "wTd": np.ascontiguousarray(W.T),
            "browd": np.ascontiguousarray(b.reshape(1, F)),
        })
    return in_maps, dict(N=N, APC=APC, T=T, P=P, out_sel=out_sel)


_NC_CACHE = {}


def kernel(**inputs) -> np.ndarray:
    n_cores = 8
    in_maps, meta = host_prep(inputs, n_cores)
    N = meta["N"]
    T = meta["T"]
    ckey = (N, T, n_cores)
    nc = _NC_CACHE.get(ckey)
    if nc is None:
        nc = build_nc(N, T, n_cores)
        _NC_CACHE[ckey] = nc
    res = run_bass_kernel_spmd(nc, in_maps, core_ids=list(range(n_cores)))
    n_sp = T // (2 * TPS)
    out = np.empty((N, 2 * F), dtype=np.float32)
    for c in range(n_cores):
        # outd [128f, (b, {V,R}, t, a)] -> slot-major rows [T*ATILE, f]
        arr = np.asarray(res.results[c]["outd"]).astype(np.float32)
        v = arr.reshape(128, n_sp, 2, 16, ATILE)
        V = v[:, :, 0].reshape(128, T * ATILE).T
        R = v[:, :, 1].reshape(128, T * ATILE).T
        rows_slot, rows_atom = meta["out_sel"][c]
        out[rows_atom, 0:F] = V[rows_slot]
        out[rows_atom, F:2 * F] = R[rows_slot]
    return out
